# revision 2
# baseline (speedup 1.0000x reference)
"""Trainium2 Bass kernel for nn_Node2Vec (EGNN message passing), 8-core SPMD, v2.

Sharding: nodes split across 8 cores (4096 each); edges assigned to the core
owning their destination (row).  Per layer the updated [h|x] rows (272 bf16:
h rounded to bf16, x as raw f32 bits) are AllGathered so column-side gathers
read a local replica.

v2 layout: h master is FEATURE-major (hT [128,2,NS] f32r) so the node MLP
needs no input transposes; a node-major bf16 copy (hxb) feeds the exchange
and the row-side gathers.  Selection matrices (node->edge selT, edge->node
sel) are precomputed on host per edge-phase config and DMA'd per 4-chunk
group.  Radial terms are batched so each layer issues one Sqrt; sigmoid is
computed via tanh so all in-layer activations share one table.
"""
import numpy as np
import ml_dtypes

NC = 8
N = 32768
NS = N // NC          # 4096 nodes per core
G = 32                # 128-node groups per core
H = 256
F = 512
VOCAB = 780
BS = 32
ROWB = 272            # bf16 exchange row: h(256) | x-bits(8) | pad(8)
N_LAYERS = 9
COORDS_RANGE = 30.0

_cache = {}
DISABLE = set()

bf16 = ml_dtypes.bfloat16


def _pack_edges(edges, cfg):
    """Assign edges to (core, chunk) slots with static window bases shared
    across cores. Returns bases plus per-core colidx/lr arrays [128, NCH]."""
    row = edges[cfg].astype(np.int64)
    col = edges[1 - cfg].astype(np.int64)

    insts = []
    percore = []
    for c in range(NC):
        m = (row // NS) == c
        r = row[m] - c * NS
        k = col[m]
        order = np.argsort(r, kind="stable")
        r, k = r[order], k[order]
        insts.append(np.bincount(r // 128, minlength=G))
        percore.append((r, k))

    def try_pack(cnt, bases):
        cap = {}
        for kk, g in enumerate(bases):
            cap.setdefault(g, []).append([kk, 128])
        for j in range(G):
            left = int(cnt[j])
            for g in (j - 1, j):
                for slot in cap.get(g, []):
                    t = min(slot[1], left)
                    slot[1] -= t
                    left -= t
                    if left == 0:
                        break
                if left == 0:
                    break
            if left > 0:
                return j
        return -1

    bases = sorted(min(j, 30) for j in range(31))
    for _ in range(200):
        bad = -1
        for cnt in insts:
            rbad = try_pack(cnt, bases)
            if rbad >= 0:
                bad = rbad
                break
        if bad < 0:
            break
        bases.append(min(bad, 30))
        bases.sort()
    else:
        raise RuntimeError("edge packing failed")

    NCH = len(bases)
    colidx = np.zeros((NC, 128, NCH), np.int64)
    lr = np.full((NC, 128, NCH), 300, np.int32)
    for c in range(NC):
        r, k = percore[c]
        grp = r // 128
        cap = {}
        for kk, g in enumerate(bases):
            cap.setdefault(g, []).append([kk, 0])
        for j in range(G):
            idxs = np.nonzero(grp == j)[0]
            pos = 0
            for g in (j - 1, j):
                for slot in cap.get(g, []):
                    while slot[1] < 128 and pos < len(idxs):
                        e = idxs[pos]
                        colidx[c, slot[1], slot[0]] = k[e]
                        lr[c, slot[1], slot[0]] = r[e] - g * 128
                        slot[1] += 1
                        pos += 1
                    if pos == len(idxs):
                        break
                if pos == len(idxs):
                    break
            assert pos == len(idxs), "packing inconsistency"
    return bases, colidx, lr


def _permrow(n):
    """DRAM p-major row index for global node id n."""
    return (n // NS) * NS + (n % 128) * G + (n % NS) // 128


def _prep(inputs):
    f32 = np.float32
    i32 = np.int32
    feature = np.asarray(inputs["feature"], f32).reshape(N, F)
    v = np.asarray(inputs["v"]).astype(i32).reshape(N)
    size = np.asarray(inputs["size"]).astype(i32).reshape(N)
    pos = np.asarray(inputs["pos"], f32).reshape(N, 3)
    edges = np.asarray(inputs["edges"]).astype(np.int64)
    predict_idx = np.asarray(inputs["predict_idx"]).astype(np.int64)
    val = np.asarray(inputs["val"], f32)

    bases0, colidx0, lr0 = _pack_edges(edges, 0)
    bases1, colidx1, lr1 = _pack_edges(edges, 1)
    NCH = max(len(bases0), len(bases1))

    def padcfg(bases, colidx, lr):
        k = NCH - len(bases)
        if k:
            bases = [0] * k + list(bases)
            colidx = np.concatenate([np.zeros((NC, 128, k), np.int64), colidx], 2)
            lr = np.concatenate([np.full((NC, 128, k), 300, i32), lr], 2)
        return bases, colidx, lr

    bases0, colidx0, lr0 = padcfg(bases0, colidx0, lr0)
    bases1, colidx1, lr1 = padcfg(bases1, colidx1, lr1)
    meta = dict(NCH=NCH, bases=(bases0, bases1))

    colperm0 = _permrow(colidx0).astype(i32)
    colperm1 = _permrow(colidx1).astype(i32)

    def selmats(lrc):
        """lrc [128, NCH] -> selT [128, NCH*2*128], selb [128, NCH*256]."""
        iota = np.arange(128)
        # selT[p, k, hh, e] = (lrc[e, k] == hh*128 + p)
        selT = (lrc.T[None, :, None, :] == (iota[:, None, None, None]
                + 128 * np.arange(2)[None, None, :, None])).astype(f32)
        # selb[e, k, n] = (lrc[e, k] == n)
        selb = (lrc[:, :, None] == np.arange(256)[None, None, :]).astype(f32)
        selT = selT.reshape(128, NCH, 256)
        both = np.concatenate([selT, selb], 2)  # [128, NCH, 512]
        return (np.ascontiguousarray(selT.reshape(128, NCH * 256)),
                np.ascontiguousarray(both.reshape(128, NCH * 512)))

    maps = []
    for c in range(NC):
        sl = slice(c * NS, (c + 1) * NS)
        featT = np.ascontiguousarray(feature[sl].T)           # [512, 4096]
        pos_pm = np.zeros((128, G, 4), f32)                   # p-major
        pos_pm[:, :, :3] = pos[sl].reshape(G, 128, 3).transpose(1, 0, 2)
        vi = np.ascontiguousarray(v[sl].reshape(G, 128).T.astype(i32))
        si = np.ascontiguousarray(size[sl].reshape(G, 128).T.astype(i32))
        st0, sb0 = selmats(lr0[c])
        st1, sb1 = selmats(lr1[c])
        nloc = np.arange(4) * 1024 + predict_idx[4 * c:4 * c + 4]
        ploc = ((nloc % 128) * G + nloc // 128).astype(i32).reshape(4, 1)
        maps.append(dict(
            featT=featT, pos_pm=np.ascontiguousarray(pos_pm.reshape(128, G * 4)),
            v_idx=vi, s_idx=si,
            colidx0=np.ascontiguousarray(colperm0[c]),
            colidx1=np.ascontiguousarray(colperm1[c]),
            selT0=st0, selT1=st1,
            selall0=sb0.astype(bf16), selall1=sb1.astype(bf16),
            pidx=ploc,
            valrow=np.ascontiguousarray(val[4 * c:4 * c + 4].reshape(1, 4)),
        ))

    def wT(x):
        return np.ascontiguousarray(np.asarray(x, f32))

    def bias2(b, nch):
        return np.ascontiguousarray(np.asarray(b, f32).reshape(nch, 128).T)

    We1 = np.asarray(inputs["We1"], f32)
    shared = dict(
        v_emb=wT(inputs["v_emb"]), size_emb=wT(inputs["size_emb"]),
        fW1=wT(inputs["fW1"]), fW2=wT(inputs["fW2"]),
        pW1=wT(inputs["pW1"]), pW2=wT(inputs["pW2"]), pW3=wT(inputs["pW3"]),
        fb1=bias2(inputs["fb1"], 2), fb2=bias2(inputs["fb2"], 2),
        pb1=bias2(inputs["pb1"], 6), pb2=bias2(inputs["pb2"], 2),
        pb3=bias2(inputs["pb3"], 2),
        We1m=np.ascontiguousarray(We1[:, :512, :]),
        We1rs=np.ascontiguousarray(We1[:, 512:514, :].sum(1, keepdims=True)),
        We2=wT(inputs["We2"]),
        Wn1=wT(inputs["Wn1"]), Wn2=wT(inputs["Wn2"]), Wc1=wT(inputs["Wc1"]),
        Wn1b=np.ascontiguousarray(np.asarray(inputs["Wn1"], np.float32)[:, 256:, :]).astype(bf16),
        be1=np.stack([bias2(np.asarray(inputs["be1"])[l], 2) for l in range(9)]),
        be2=np.stack([bias2(np.asarray(inputs["be2"])[l], 2) for l in range(9)]),
        bn1=np.stack([bias2(np.asarray(inputs["bn1"])[l], 2) for l in range(9)]),
        bn2=np.stack([bias2(np.asarray(inputs["bn2"])[l], 2) for l in range(9)]),
        bc1row=np.ascontiguousarray(np.asarray(inputs["bc1"], f32).reshape(9, 1, H)),
        wc2row=np.ascontiguousarray(np.asarray(inputs["Wc2"], f32).transpose(0, 2, 1)),
        wattc=wT(inputs["Watt"]),                       # [9, 256, 1]
        wc2c=wT(inputs["Wc2"]),                         # [9, 256, 1]
        batth=np.ascontiguousarray(np.broadcast_to(
            0.5 * np.asarray(inputs["batt"], f32).reshape(9, 1, 1),
            (9, 128, 1)).copy()),
        oW1=wT(inputs["oW1"]), oW2=wT(inputs["oW2"]),
        ob1=bias2(inputs["ob1"], 2),
        ob2=np.ascontiguousarray(
            np.pad(np.asarray(inputs["ob2"], f32), (0, 128 * 7 - VOCAB)).reshape(7, 128).T),
        ones_row=np.ones((1, 128), f32),
    )
    for m in maps:
        m.update(shared)
    return meta, maps


def _build(meta, nl=N_LAYERS, with_head=True, dbg=(), sim1=False):
    import concourse.bacc as bacc
    import concourse.bass as bass
    import concourse.mybir as mybir
    import concourse.tile as tile
    from concourse.masks import make_identity

    dt = mybir.dt
    AF = mybir.ActivationFunctionType
    ALU = mybir.AluOpType
    NCH = meta["NCH"]
    BASES = meta["bases"]
    NST = (NCH + 3) // 4

    nc = bacc.Bacc("TRN2", target_bir_lowering=False, debug=False,
                   num_devices=1 if sim1 else NC, enable_asserts=False)

    def din(name, shape, d=dt.float32):
        return nc.dram_tensor(name, list(shape), d, kind="ExternalInput")

    featT = din("featT", [F, NS], dt.float32r)
    pos_pm = din("pos_pm", [128, G * 4])
    v_idx = din("v_idx", [128, G], dt.int32)
    s_idx = din("s_idx", [128, G], dt.int32)
    colidx_t = [din("colidx0", [128, NCH], dt.int32), din("colidx1", [128, NCH], dt.int32)]
    selT_t = [din("selT0", [128, NCH * 256], dt.float32r),
              din("selT1", [128, NCH * 256], dt.float32r)]
    selall_t = [din("selall0", [128, NCH * 512], dt.bfloat16),
                din("selall1", [128, NCH * 512], dt.bfloat16)]
    pidx = din("pidx", [4, 1], dt.int32)
    valrow = din("valrow", [1, 4])
    v_emb = din("v_emb", [VOCAB + 1, H], dt.float32r)
    size_emb = din("size_emb", [26, H], dt.float32r)
    fW1 = din("fW1", [F, H], dt.float32r); fW2 = din("fW2", [H, H], dt.float32r)
    pW1 = din("pW1", [3 * H, 3 * H], dt.float32r); pW2 = din("pW2", [3 * H, H], dt.float32r); pW3 = din("pW3", [H, H], dt.float32r)
    fb1 = din("fb1", [128, 2]); fb2 = din("fb2", [128, 2])
    pb1 = din("pb1", [128, 6]); pb2 = din("pb2", [128, 2]); pb3 = din("pb3", [128, 2])
    We1m = din("We1m", [9, 512, H], dt.float32r)
    We1rs = din("We1rs", [9, 1, H], dt.float32r)
    We2 = din("We2", [9, H, H], dt.float32r)
    Wn1 = din("Wn1", [9, 2 * H, H], dt.float32r); Wn2 = din("Wn2", [9, H, H], dt.float32r); Wc1 = din("Wc1", [9, H, H], dt.float32r)
    Wn1b = din("Wn1b", [9, H, H], dt.bfloat16)
    be1 = din("be1", [9, 128, 2]); be2 = din("be2", [9, 128, 2])
    bn1 = din("bn1", [9, 128, 2]); bn2 = din("bn2", [9, 128, 2])
    bc1row = din("bc1row", [9, 1, H])
    wc2row = din("wc2row", [9, 1, H])
    wattc_d = din("wattc", [9, H, 1], dt.float32)
    wc2c_d = din("wc2c", [9, H, 1], dt.float32)
    batth_d = din("batth", [9, 128, 1])
    oW1 = din("oW1", [H + 1, H]); oW2 = din("oW2", [H, VOCAB])
    ob1 = din("ob1", [128, 2]); ob2 = din("ob2", [128, 7])
    ones_row = din("ones_row", [1, 128])

    head_out = nc.dram_tensor("head_out", [4, VOCAB], dt.float32, kind="ExternalOutput")
    dbg_out = {}
    for name in dbg:
        dbg_out[name] = nc.dram_tensor(f"dbg_{name}", [128, 2 * NS + G * 4],
                                       dt.float32, kind="ExternalOutput")

    with tile.TileContext(nc) as tc:
        import contextlib
        ctx = contextlib.ExitStack()
        with ctx:
            pers = ctx.enter_context(tc.tile_pool(name="pers", bufs=1))
            sb = ctx.enter_context(tc.tile_pool(name="sb", bufs=2))
            ps = ctx.enter_context(tc.tile_pool(name="ps", bufs=4, space="PSUM"))
            psacc = ctx.enter_context(tc.tile_pool(name="psacc", bufs=2, space="PSUM"))
            psB = ctx.enter_context(tc.tile_pool(name="psB", bufs=2, space="PSUM"))
            dram = ctx.enter_context(tc.tile_pool(name="dram", bufs=1, space="DRAM"))

            bounce = dram.tile([128, G, ROWB], dt.bfloat16)

            hT = pers.tile([128, 2, NS], dt.float32r)
            hxb = pers.tile([128, G, ROWB], dt.bfloat16)
            xnode = pers.tile([128, G, 4], dt.float32r)
            aggT = pers.tile([128, 2, NS], dt.bfloat16)
            xacc = pers.tile([128, G, 4], dt.float32)
            cgall = pers.tile([128, NCH, ROWB], dt.bfloat16)
            cdg = pers.tile([128, NCH, 4], dt.float32r)
            radcol = pers.tile([128, NCH], dt.float32)

            nc.gpsimd.memset(aggT[:], 0.0)
            nc.gpsimd.memset(xacc[:], 0.0)
            ident = pers.tile([128, 128], dt.float32)
            make_identity(nc, ident[:])
            identb = pers.tile([128, 128], dt.bfloat16)
            nc.vector.tensor_copy(identb[:], ident[:])
            identr = pers.tile([128, 128], dt.float32r)
            nc.vector.tensor_copy(identr[:], ident[:])
            onesr = pers.tile([1, 128], dt.float32)
            nc.sync.dma_start(onesr[:], ones_row[:])
            vidxt = pers.tile([128, G], dt.int32)
            nc.sync.dma_start(vidxt[:], v_idx[:])
            sidxt = pers.tile([128, G], dt.int32)
            nc.sync.dma_start(sidxt[:], s_idx[:])
            colt = [pers.tile([128, NCH], dt.int32, tag=f"colt{i}", name=f"colt{i}")
                    for i in range(2)]
            nc.sync.dma_start(colt[0][:], colidx_t[0][:])
            nc.sync.dma_start(colt[1][:], colidx_t[1][:])

            def mm(out, lhsT, rhs, start, stop):
                nc.tensor.matmul(out=out, lhsT=lhsT, rhs=rhs, start=start, stop=stop)

            def act(out, in_, func, bias=0.0, scale=1.0):
                nc.scalar.activation(out, in_, func, bias=bias, scale=scale)

            # round-robin PSUM->SBUF copy engine balancing
            _cp_state = [0]

            def cpbal(dst, src):
                i = _cp_state[0] = (_cp_state[0] + 1) % 2
                if i == 0:
                    nc.vector.tensor_copy(dst, src)
                else:
                    nc.scalar.copy(dst, src)

            # ============ embedding ============
            with tc.tile_pool(name="embw", bufs=1) as embw, \
                 tc.tile_pool(name="embs", bufs=1) as embs:
                xtmp = embs.tile([128, G * 4], dt.float32, tag="xtmp", name="xtmp")
                nc.sync.dma_start(xtmp[:], pos_pm[:])
                nc.vector.tensor_copy(xnode[:],
                                      xtmp[:].rearrange("p (g m) -> p g m", m=4))
                nc.vector.tensor_copy(
                    hxb[:, :, 256:264],
                    xtmp[:].rearrange("p (g m) -> p g m", m=4).bitcast(dt.bfloat16))

                def loadw(pool, src, kch, m_, tag):
                    t = pool.tile([128, kch, m_], dt.float32r, tag=tag, name=tag)
                    nc.sync.dma_start(t[:], src[:].rearrange("(k p) m -> p k m", p=128))
                    return t

                fW1t = loadw(embw, fW1, 4, H, "fW1")
                fW2t = loadw(embw, fW2, 2, H, "fW2")
                pW1t = loadw(embw, pW1, 6, 3 * H, "pW1")
                pW2t = loadw(embw, pW2, 6, H, "pW2")
                pW3t = loadw(embw, pW3, 2, H, "pW3")
                bt = {}
                for nm, src, w in (("fb1", fb1, 2), ("fb2", fb2, 2), ("pb1", pb1, 6),
                                   ("pb2", pb2, 2), ("pb3", pb3, 2)):
                    bt[nm] = embw.tile([128, w], dt.float32, tag=nm, name=nm)
                    nc.sync.dma_start(bt[nm][:], src[:])

                for b in range(8):
                    bsl = slice(b * 512, (b + 1) * 512)
                    fe1p = [psacc.tile([128, 512], dt.float32, tag="acc", name="acc") for _ in range(2)]
                    for k in range(4):
                        ft = embs.tile([128, 512], dt.float32r, tag="ft", name="ft")
                        nc.sync.dma_start(ft[:], featT[k * 128:(k + 1) * 128, bsl])
                        for m_ in range(2):
                            mm(fe1p[m_][:], fW1t[:, k, m_ * 128:(m_ + 1) * 128], ft[:],
                               k == 0, k == 3)
                    fe1 = [embs.tile([128, 512], dt.float32r, tag=f"fe1_{i}", name=f"fe1_{i}") for i in range(2)]
                    for m_ in range(2):
                        act(fe1[m_][:], fe1p[m_][:], AF.Silu, bias=bt["fb1"][:, m_:m_ + 1])
                    fe2p = [psacc.tile([128, 512], dt.float32, tag="acc", name="acc") for _ in range(2)]
                    for k in range(2):
                        for m_ in range(2):
                            mm(fe2p[m_][:], fW2t[:, k, m_ * 128:(m_ + 1) * 128], fe1[k][:],
                               k == 0, k == 1)
                    comb6 = embs.tile([128, 6, 512], dt.float32r, tag="comb6", name="comb6")
                    for m_ in range(2):
                        act(comb6[:, 2 + m_, :], fe2p[m_][:], AF.Identity,
                            bias=bt["fb2"][:, m_:m_ + 1])
                    gts = []
                    for idxt, off in ((vidxt, 0), (sidxt, 4)):
                        tbl = v_emb if off == 0 else size_emb
                        for j in range(4):
                            g = b * 4 + j
                            gt = embs.tile([128, H], dt.float32r, tag="embrow", name="embrow", bufs=8)
                            nc.gpsimd.indirect_dma_start(
                                out=gt[:], out_offset=None, in_=tbl[:],
                                in_offset=bass.IndirectOffsetOnAxis(
                                    ap=idxt[:, g:g + 1], axis=0))
                            gts.append((gt, off, j))
                    for gt, off, j in gts:
                        gtp = ps.tile([128, 2, 128], dt.float32r, tag="small", name="gtp")
                        for m_ in range(2):
                            nc.tensor.transpose(out=gtp[:, m_, :],
                                                in_=gt[:, m_ * 128:(m_ + 1) * 128],
                                                identity=identr[:])
                        co = 0 if off == 0 else 4
                        cpbal(comb6[:, co:co + 2, j * 128:(j + 1) * 128], gtp[:])
                    hp2p = [psB.tile([128, 512], dt.float32, tag="psB", name="hp2p") for _ in range(2)]
                    for mo in range(6):
                        hp1p = psacc.tile([128, 512], dt.float32, tag="acc", name="hp1p")
                        for k in range(6):
                            mm(hp1p[:], pW1t[:, k, mo * 128:(mo + 1) * 128],
                               comb6[:, k, :], k == 0, k == 5)
                        hp1t = embs.tile([128, 512], dt.float32r, tag="hp1t", name="hp1t")
                        act(hp1t[:], hp1p[:], AF.Silu, bias=bt["pb1"][:, mo:mo + 1])
                        for m_ in range(2):
                            mm(hp2p[m_][:], pW2t[:, mo, m_ * 128:(m_ + 1) * 128], hp1t[:],
                               mo == 0, mo == 5)
                    hp2 = [embs.tile([128, 512], dt.float32r, tag=f"hp2_{i}", name=f"hp2_{i}") for i in range(2)]
                    for m_ in range(2):
                        act(hp2[m_][:], hp2p[m_][:], AF.Silu, bias=bt["pb2"][:, m_:m_ + 1])
                    h0p = [psacc.tile([128, 512], dt.float32, tag="acc", name="acc") for _ in range(2)]
                    for k in range(2):
                        for m_ in range(2):
                            mm(h0p[m_][:], pW3t[:, k, m_ * 128:(m_ + 1) * 128], hp2[k][:],
                               k == 0, k == 1)
                    for m_ in range(2):
                        act(hT[:, m_, bsl], h0p[m_][:], AF.Identity,
                            bias=bt["pb3"][:, m_:m_ + 1])
                        etp = psB.tile([128, 4, 128], dt.float32r, tag="psB", name="etp")
                        for j in range(4):
                            nc.tensor.transpose(
                                out=etp[:, j, :],
                                in_=hT[:, m_, b * 512 + j * 128:b * 512 + (j + 1) * 128],
                                identity=identr[:])
                        cpbal(hxb[:, 4 * b:4 * b + 4, m_ * 128:(m_ + 1) * 128], etp[:])
                    nc.sync.dma_start(bounce[:, 4 * b:4 * b + 4, :],
                                      hxb[:, 4 * b:4 * b + 4, :])

            # ============ GCL layers ============
            wpool = ctx.enter_context(tc.tile_pool(name="wpool", bufs=1))
            sb2 = ctx.enter_context(tc.tile_pool(name="sb2", bufs=2))

            def dump_state(name):
                if name in dbg_out:
                    d = dbg_out[name]
                    nc.sync.dma_start(
                        d[:, :2 * NS].rearrange("p (a n) -> p a n", a=2),
                        hT[:].bitcast(dt.float32))
                    nc.sync.dma_start(
                        d[:, 2 * NS:].rearrange("p (g m) -> p g m", m=4),
                        xnode[:].bitcast(dt.float32))

            for l in range(nl):
                cfg = 0 if (l // 3) % 2 == 0 else 1
                bases = BASES[cfg]
                dump_state(f"s{l}")

                if sim1:
                    hx_full = dram.tile([NC * 128, G, ROWB], dt.bfloat16,
                                        tag="hxsim", name="hxsim")
                    nc.sync.dma_start(hx_full[0:128, :, :], bounce[:])
                else:
                    hx_full = dram.tile([NC * 128, G, ROWB], dt.bfloat16,
                                        addr_space="Shared", tag=f"hx{l}", name=f"hx{l}")
                hx_rows = hx_full[:].rearrange("p g m -> (p g) m")
                if not sim1:
                    nc.gpsimd.collective_compute(
                        "AllGather", mybir.AluOpType.bypass,
                        replica_groups=[list(range(NC))],
                        ins=[bounce.opt()], outs=[hx_full.opt()])

                # --- layer weights ---
                We1t = wpool.tile([128, 4, H], dt.float32r, tag="We1", name="We1")
                nc.sync.dma_start(We1t[:], We1m[l][:].rearrange("(k p) m -> p k m", p=128))
                We1rt = wpool.tile([1, H], dt.float32r, tag="We1r", name="We1r")
                nc.sync.dma_start(We1rt[:], We1rs[l][:])
                We2t = wpool.tile([128, 2, H], dt.float32r, tag="We2", name="We2")
                nc.sync.dma_start(We2t[:], We2[l][:].rearrange("(k p) m -> p k m", p=128))
                Wn1t = wpool.tile([128, 4, H], dt.float32r, tag="Wn1", name="Wn1")
                nc.sync.dma_start(Wn1t[:], Wn1[l][:].rearrange("(k p) m -> p k m", p=128))
                Wn1bt = wpool.tile([128, 2, H], dt.bfloat16, tag="Wn1b", name="Wn1b")
                nc.sync.dma_start(Wn1bt[:], Wn1b[l][:].rearrange("(k p) m -> p k m", p=128))
                Wn2t = wpool.tile([128, 2, H], dt.float32r, tag="Wn2", name="Wn2")
                nc.sync.dma_start(Wn2t[:], Wn2[l][:].rearrange("(k p) m -> p k m", p=128))
                Wc1t = wpool.tile([128, 2, H], dt.float32r, tag="Wc1", name="Wc1")
                nc.sync.dma_start(Wc1t[:], Wc1[l][:].rearrange("(k p) m -> p k m", p=128))
                wattt = wpool.tile([128, 2, 1], dt.float32, tag="watt", name="watt")
                nc.sync.dma_start(wattt[:], wattc_d[l][:].rearrange("(k p) m -> p k m", p=128))
                wc2t = wpool.tile([128, 2, 1], dt.float32, tag="wc2", name="wc2")
                nc.sync.dma_start(wc2t[:], wc2c_d[l][:].rearrange("(k p) m -> p k m", p=128))
                lb = {}
                for nm, src in (("be1", be1), ("be2", be2), ("bn1", bn1), ("bn2", bn2)):
                    lb[nm] = wpool.tile([128, 2], dt.float32, tag=f"l{nm}", name=f"l{nm}")
                    nc.sync.dma_start(lb[nm][:], src[l][:])
                batht = wpool.tile([128, 1], dt.float32, tag="bath", name="bath")
                nc.sync.dma_start(batht[:], batth_d[l][:])

                def bcast_row(src, tag):
                    r_ = wpool.tile([1, H], dt.float32, tag=tag + "r", name=tag + "r")
                    nc.sync.dma_start(r_[:], src[l][:])
                    p_ = ps.tile([128, H], dt.float32, tag="small", name="small")
                    mm(p_[:], onesr[:], r_[:], True, True)
                    t_ = wpool.tile([128, H], dt.float32, tag=tag, name=tag)
                    nc.scalar.copy(t_[:], p_[:])
                    return t_

                bc1b = bcast_row(bc1row, "bc1b")
                wc2b = bcast_row(wc2row, "wc2b")

                nc.vector.tensor_scalar_mul(out=aggT[:], in0=aggT[:], scalar1=0.0)
                nc.vector.tensor_scalar_mul(out=xacc[:], in0=xacc[:], scalar1=0.0)

                # --- A0: column gathers ---
                for k in range(NCH if "A0" not in DISABLE else 0):
                    nc.gpsimd.indirect_dma_start(
                        out=cgall[:, k, :], out_offset=None, in_=hx_rows,
                        in_offset=bass.IndirectOffsetOnAxis(
                            ap=colt[cfg][:, k:k + 1], axis=0))

                # --- A2/B/C per group ---
                for st in range(NST if "A2BC" not in DISABLE else 0):
                    ch0 = st * 4
                    nch_st = min(4, NCH - ch0)
                    W = nch_st * 128
                    sall = sb2.tile([128, 4, 512], dt.bfloat16, tag="sall", name="sall")
                    nc.sync.dma_start(
                        sall[:, :nch_st, :],
                        selall_t[cfg][:, ch0 * 512: ch0 * 512 + nch_st * 512]
                        .rearrange("p (c n) -> p c n", n=512))
                    # --- A1: row x, diff, radial, 30/(1+sqrt(r)) via DVE rsqrt ---
                    sTf = sb2.tile([128, 8, 128], dt.float32r, tag="sTf", name="sTf")
                    nc.sync.dma_start(
                        sTf[:, :2 * nch_st, :],
                        selT_t[cfg][:, ch0 * 256: ch0 * 256 + nch_st * 256]
                        .rearrange("p (c e) -> p c e", e=128))
                    xrps = ps.tile([128, 4, 4], dt.float32, tag="small", name="xrps")
                    for c in range(nch_st):
                        gb = bases[ch0 + c]
                        for hh in range(2):
                            mm(xrps[:, c, :], sTf[:, 2 * c + hh, :],
                               xnode[:, gb + hh, :], hh == 0, hh == 1)
                    csl = slice(ch0, ch0 + nch_st)
                    nc.vector.tensor_tensor(
                        out=cdg[:, csl, :], in0=xrps[:, :nch_st, :],
                        in1=cgall[:, csl, 256:264].bitcast(dt.float32),
                        op=ALU.subtract)
                    sqg = sb.tile([128, 4, 3], dt.float32, tag="sqg", name="sqg")
                    nc.vector.tensor_tensor(out=sqg[:, :nch_st, :],
                                            in0=cdg[:, csl, 0:3],
                                            in1=cdg[:, csl, 0:3], op=ALU.mult)
                    nc.vector.tensor_reduce(
                        out=radcol[:, csl].rearrange("p (a o) -> p a o", o=1),
                        in_=sqg[:, :nch_st, :], axis=mybir.AxisListType.X, op=ALU.add)
                    nsl = slice(0, nch_st)
                    rclp = sb.tile([128, 4], dt.float32, tag="rclp", name="rclp")
                    nc.vector.tensor_scalar_max(out=rclp[:, nsl],
                                                in0=radcol[:, csl], scalar1=1e-12)
                    yq = sb.tile([128, 4], dt.int32, tag="yq", name="yq")
                    nc.vector.tensor_scalar(out=yq[:, nsl],
                                            in0=rclp[:, nsl].bitcast(dt.int32),
                                            scalar1=1, scalar2=None,
                                            op0=ALU.logical_shift_right)
                    nc.vector.tensor_scalar(out=yq[:, nsl], in0=yq[:, nsl],
                                            scalar1=-1, scalar2=0x5f3759df,
                                            op0=ALU.mult, op1=ALU.add)
                    yf = yq[:].bitcast(dt.float32)
                    tn = sb.tile([128, 4], dt.float32, tag="tn", name="tn")
                    for _ in range(2):
                        nc.vector.tensor_tensor(out=tn[:, nsl], in0=yf[:, nsl],
                                                in1=yf[:, nsl], op=ALU.mult)
                        nc.vector.tensor_tensor(out=tn[:, nsl], in0=tn[:, nsl],
                                                in1=rclp[:, nsl], op=ALU.mult)
                        nc.vector.tensor_scalar(out=tn[:, nsl], in0=tn[:, nsl],
                                                scalar1=-0.5, scalar2=1.5,
                                                op0=ALU.mult, op1=ALU.add)
                        nc.vector.tensor_tensor(out=yf[:, nsl], in0=yf[:, nsl],
                                                in1=tn[:, nsl], op=ALU.mult)
                    # den = 1 + sqrt(r) = 1 + r * rsqrt(r); y/(1+r*y) via 2nd hack pass
                    nc.vector.tensor_tensor(out=tn[:, nsl], in0=rclp[:, nsl],
                                            in1=yf[:, nsl], op=ALU.mult)
                    nc.vector.tensor_scalar_add(out=tn[:, nsl], in0=tn[:, nsl],
                                                scalar1=1.0)
                    rec4 = sb.tile([128, 4], dt.float32, tag="rec4", name="rec4")
                    nc.vector.reciprocal(rec4[:, nsl], tn[:, nsl])
                    rc30g = sb.tile([128, 4, 1], dt.float32, tag="rc30g", name="rc30g")
                    nc.vector.tensor_scalar_mul(
                        out=rc30g[:, nsl, :],
                        in0=rec4[:, nsl].rearrange("p (a o) -> p a o", o=1),
                        scalar1=float(COORDS_RANGE))
                    nc.vector.tensor_tensor(out=cdg[:, csl, :], in0=cdg[:, csl, :],
                                            in1=rc30g[:, nsl, :].to_broadcast(
                                                [128, nch_st, 4]),
                                            op=ALU.mult)

                    efT4 = sb2.tile([128, 4, 512], dt.float32r, tag="efT4", name="efT4", bufs=2)
                    efr1 = sb2.tile([1, 512], dt.float32r, tag="efr1", name="efr1")
                    tpbs, rpps, rtps = [], [], []
                    for c in range(nch_st):
                        k = ch0 + c
                        tpb = ps.tile([128, 2, 128], dt.bfloat16, tag="small", name="tpb")
                        for m_ in range(2):
                            nc.tensor.transpose(
                                out=tpb[:, m_, :], in_=cgall[:, k, m_ * 128:(m_ + 1) * 128],
                                identity=identb[:])
                        tpbs.append(tpb)
                    for c in range(nch_st):
                        jsl = slice(c * 128, (c + 1) * 128)
                        cpbal(efT4[:, 2:4, jsl], tpbs[c][:])
                    for c in range(nch_st):
                        gb = bases[ch0 + c]
                        rpp = ps.tile([128, 2, 128], dt.float32, tag="small", name="rpp")
                        for m_ in range(2):
                            msl = slice(m_ * 128, (m_ + 1) * 128)
                            for hh in range(2):
                                mm(rpp[:, m_, :], hxb[:, gb + hh, msl],
                                   sall[:, c, hh * 128:(hh + 1) * 128],
                                   hh == 0, hh == 1)
                        rpps.append(rpp)
                    for c in range(nch_st):
                        jsl = slice(c * 128, (c + 1) * 128)
                        cpbal(efT4[:, 0:2, jsl], rpps[c][:])
                    for c in range(nch_st):
                        k = ch0 + c
                        rtp = ps.tile([1, 128], dt.float32, tag="small", name="rtp")
                        nc.tensor.transpose(out=rtp[:], in_=radcol[:, k:k + 1],
                                            identity=ident[:])
                        rtps.append(rtp)
                    for c in range(nch_st):
                        jsl = slice(c * 128, (c + 1) * 128)
                        nc.scalar.copy(efr1[0:1, jsl], rtps[c][:])

                    # --- B: edge MLP ---
                    if "B" in DISABLE:
                        continue
                    m1p = [psacc.tile([128, 512], dt.float32, tag="acc", name="acc")
                           for _ in range(2)]
                    for m_ in range(2):
                        msl = slice(m_ * 128, (m_ + 1) * 128)
                        for k in range(4):
                            mm(m1p[m_][:, :W], We1t[:, k, msl], efT4[:, k, :W],
                               k == 0, False)
                        mm(m1p[m_][:, :W], We1rt[0:1, msl], efr1[0:1, :W], False, True)
                    msg1 = [sb2.tile([128, 512], dt.float32r, tag=f"msg1_{i}", name=f"msg1_{i}", bufs=2)
                            for i in range(2)]
                    for m_ in range(2):
                        act(msg1[m_][:, :W], m1p[m_][:, :W], AF.Silu,
                            bias=lb["be1"][:, m_:m_ + 1])
                    m2p = [psacc.tile([128, 512], dt.float32, tag="acc", name="acc")
                           for _ in range(2)]
                    for m_ in range(2):
                        msl = slice(m_ * 128, (m_ + 1) * 128)
                        for k in range(2):
                            mm(m2p[m_][:, :W], We2t[:, k, msl], msg1[k][:, :W],
                               k == 0, k == 1)
                    msg2 = [sb2.tile([128, 512], dt.float32r, tag=f"msg2_{i}", name=f"msg2_{i}")
                            for i in range(2)]
                    for m_ in range(2):
                        act(msg2[m_][:, :W], m2p[m_][:, :W], AF.Silu,
                            bias=lb["be2"][:, m_:m_ + 1])

                    # --- C: attention, aggregation, coordinate path ---
                    if "Catt" in DISABLE:
                        continue
                    attps = ps.tile([128, 4], dt.float32, tag="small", name="attps")
                    for c in range(nch_st if "CattMM" not in DISABLE else 0):
                        jsl = slice(c * 128, (c + 1) * 128)
                        for k in range(2):
                            mm(attps[:, c:c + 1],
                               msg2[k][:, jsl].bitcast(dt.float32),
                               wattt[:, k, :], k == 0, k == 1)
                    atts01 = sb.tile([128, 4], dt.float32, tag="atts", name="atts")
                    if "CattAct" in DISABLE:
                        nc.vector.tensor_copy(atts01[:, :nch_st], attps[:, :nch_st])
                    else:
                        act(atts01[:, :nch_st], attps[:, :nch_st], AF.Tanh,
                            bias=batht[:, 0:1], scale=0.5)
                    if "CattTS" not in DISABLE:
                        nc.vector.tensor_scalar(out=atts01[:, :nch_st],
                                            in0=atts01[:, :nch_st],
                                            scalar1=0.5, scalar2=0.5,
                                            op0=ALU.mult, op1=ALU.add)
                    if "Cloop" in DISABLE:
                        continue
                    zcol = sb.tile([128, 4], dt.float32, tag="zcol", name="zcol")
                    mtps, msges, us, tes = [], [], [], []
                    for c in range(nch_st):
                        jsl = slice(c * 128, (c + 1) * 128)
                        mtp = ps.tile([128, 2, 128], dt.float32r, tag="small", name="mtp")
                        for m_ in range(2):
                            nc.tensor.transpose(out=mtp[:, m_, :], in_=msg2[m_][:, jsl],
                                                identity=identr[:])
                        mtps.append(mtp)
                    for c in range(nch_st):
                        msge = sb.tile([128, 2, 128], dt.bfloat16, tag="msge",
                                       name="msge", bufs=5)
                        act(msge[:], mtps[c][:], AF.Copy, scale=atts01[:, c:c + 1])
                        msges.append(msge)
                    for c in range(nch_st):
                        jsl = slice(c * 128, (c + 1) * 128)
                        weps = psB.tile([128, 256], dt.float32, tag="psB", name="weps")
                        for k2 in range(2):
                            mm(weps[:], msg2[k2][:, jsl], Wc1t[:, k2, :],
                               k2 == 0, k2 == 1)
                        u = sb.tile([128, 256], dt.float32, tag="u", name="u", bufs=5)
                        nc.vector.scalar_tensor_tensor(
                            out=u[:], in0=weps[:], scalar=atts01[:, c:c + 1],
                            in1=bc1b[:], op0=ALU.mult, op1=ALU.add)
                        us.append(u)
                    for c in range(nch_st):
                        gb = bases[ch0 + c]
                        spps = psB.tile([128, 2, 256], dt.float32, tag="psB", name="spps")
                        for m_ in range(2):
                            mm(spps[:, m_, :], msges[c][:, m_, :],
                               sall[:, c, 256:512], True, True)
                        win = aggT[:, :, gb * 128:gb * 128 + 256]
                        nc.vector.tensor_tensor(out=win, in0=win, in1=spps[:],
                                                op=ALU.add)
                    for c in range(nch_st):
                        te = sb.tile([128, 256], dt.float32, tag="te", name="te", bufs=5)
                        act(te[:], us[c][:], AF.Silu)
                        tes.append(te)
                    for c in range(nch_st):
                        zm = sb.tile([128, 256], dt.float32, tag="zm", name="zm")
                        nc.vector.scalar_tensor_tensor(
                            out=zm[:], in0=tes[c][:], scalar=1.0, in1=wc2b[:],
                            op0=ALU.mult, op1=ALU.mult,
                            accum_out=zcol[:, c:c + 1])
                    if "Cth" in DISABLE:
                        continue
                    if "Cth" in DISABLE:
                        continue
                    thg = sb.tile([128, 4], dt.float32, tag="thg", name="thg")
                    act(thg[:, :nch_st], zcol[:, :nch_st], AF.Tanh)
                    cdt = sb.tile([128, 4, 4], dt.bfloat16, tag="cdt", name="cdt")
                    nc.vector.tensor_tensor(
                        out=cdt[:, :nch_st, :], in0=cdg[:, ch0:ch0 + nch_st, :],
                        in1=thg[:, :nch_st].rearrange("p (a o) -> p a o", o=1)
                        .to_broadcast([128, nch_st, 4]),
                        op=ALU.mult)
                    for c in range(nch_st):
                        gb = bases[ch0 + c]
                        xpps = ps.tile([128, 2, 4], dt.float32, tag="small", name="xpps")
                        for hh in range(2):
                            mm(xpps[:, hh, :],
                               sall[:, c, 256 + hh * 128:256 + (hh + 1) * 128],
                               cdt[:, c, :], True, True)
                        xwin = xacc[:, gb:gb + 2, :]
                        nc.vector.tensor_tensor(out=xwin, in0=xwin, in1=xpps[:],
                                                op=ALU.add)

                # --- x update ---
                nc.vector.tensor_tensor(out=xnode[:], in0=xnode[:],
                                        in1=xacc[:].bitcast(dt.float32r), op=ALU.add)
                nc.vector.tensor_copy(hxb[:, :, 256:264], xnode[:].bitcast(dt.bfloat16))

                # --- node MLP (feature-major) ---
                for b in range(8 if "node" not in DISABLE else 0):
                    bsl = slice(b * 512, (b + 1) * 512)
                    n1p = [psacc.tile([128, 512], dt.float32, tag="acc", name="acc")
                           for _ in range(2)]
                    for m_ in range(2):
                        msl = slice(m_ * 128, (m_ + 1) * 128)
                        for k in range(2):
                            mm(n1p[m_][:], Wn1t[:, k, msl], hT[:, k, bsl], k == 0, False)
                        for k in range(2):
                            mm(n1p[m_][:], Wn1bt[:, k, msl], aggT[:, k, bsl],
                               False, k == 1)
                    nh1 = [sb2.tile([128, 512], dt.float32r, tag=f"nh1_{i}", name=f"nh1_{i}", bufs=2)
                           for i in range(2)]
                    for m_ in range(2):
                        act(nh1[m_][:], n1p[m_][:], AF.Silu, bias=lb["bn1"][:, m_:m_ + 1])
                    n2p = [psacc.tile([128, 512], dt.float32, tag="acc", name="acc")
                           for _ in range(2)]
                    for m_ in range(2):
                        msl = slice(m_ * 128, (m_ + 1) * 128)
                        for k in range(2):
                            mm(n2p[m_][:], Wn2t[:, k, msl], nh1[k][:], k == 0, k == 1)
                    for m_ in range(2):
                        nc.vector.scalar_tensor_tensor(
                            out=hT[:, m_, bsl], in0=n2p[m_][:],
                            scalar=lb["bn2"][:, m_:m_ + 1], in1=hT[:, m_, bsl],
                            op0=ALU.add, op1=ALU.add)
                        ntp = psB.tile([128, 4, 128], dt.float32r, tag="psB", name="ntp")
                        for j in range(4):
                            nc.tensor.transpose(
                                out=ntp[:, j, :],
                                in_=hT[:, m_, b * 512 + j * 128:b * 512 + (j + 1) * 128],
                                identity=identr[:])
                        cpbal(hxb[:, 4 * b:4 * b + 4, m_ * 128:(m_ + 1) * 128], ntp[:])
                    nc.sync.dma_start(bounce[:, 4 * b:4 * b + 4, :],
                                      hxb[:, 4 * b:4 * b + 4, :])

            dump_state(f"s{nl}")
            bouncef = dram.tile([128, G, ROWB], dt.bfloat16, tag="bouncef", name="bouncef")
            nc.sync.dma_start(bouncef[:], hxb[:])
            bounce_rows = bouncef[:].rearrange("p g m -> (p g) m")

            # ============ output head ============
            if with_head:
                oW1t = wpool.tile([128, 2, H], dt.float32, tag="oW1", name="oW1")
                nc.sync.dma_start(oW1t[:], oW1[0:256, :].rearrange("(k p) m -> p k m", p=128))
                oW1v = wpool.tile([1, H], dt.float32, tag="oW1v", name="oW1v")
                nc.sync.dma_start(oW1v[:], oW1[256:257, :])
                oW2t = wpool.tile([128, 2, VOCAB], dt.float32, tag="oW2", name="oW2")
                nc.sync.dma_start(oW2t[:], oW2[:].rearrange("(k p) m -> p k m", p=128))
                ob1t = wpool.tile([128, 2], dt.float32, tag="ob1", name="ob1")
                nc.sync.dma_start(ob1t[:], ob1[:])
                ob2t = wpool.tile([128, 7], dt.float32, tag="ob2", name="ob2")
                nc.sync.dma_start(ob2t[:], ob2[:])
                pidxt = sb.tile([4, 1], dt.int32, tag="pidxt", name="pidxt")
                nc.sync.dma_start(pidxt[:], pidx[:])
                valt = sb.tile([1, 4], dt.float32, tag="valt", name="valt")
                nc.sync.dma_start(valt[:], valrow[:])

                hselb = sb.tile([4, ROWB], dt.bfloat16, tag="hselb", name="hselb")
                nc.gpsimd.indirect_dma_start(
                    out=hselb[:], out_offset=None, in_=bounce_rows,
                    in_offset=bass.IndirectOffsetOnAxis(ap=pidxt[:, :1], axis=0))
                hsel = sb.tile([4, H], dt.float32, tag="hsel", name="hsel")
                nc.vector.tensor_copy(hsel[:], hselb[:, 0:256])
                hselT = sb.tile([128, 2, 4], dt.float32, tag="hselT", name="hselT")
                for m_ in range(2):
                    tp = ps.tile([128, 4], dt.float32, tag="small", name="small")
                    nc.tensor.transpose(out=tp[:], in_=hsel[:, m_ * 128:(m_ + 1) * 128],
                                        identity=ident[:4, :4])
                    nc.vector.tensor_copy(hselT[:, m_, :], tp[:])
                o1p = [ps.tile([128, 4], dt.float32, tag="small", name="small") for _ in range(2)]
                for m_ in range(2):
                    msl = slice(m_ * 128, (m_ + 1) * 128)
                    for k in range(2):
                        mm(o1p[m_][:], oW1t[:, k, msl], hselT[:, k, :], k == 0, False)
                    mm(o1p[m_][:], oW1v[:, msl], valt[:], False, True)
                o1 = sb.tile([128, 2, 4], dt.float32, tag="o1", name="o1")
                for m_ in range(2):
                    act(o1[:, m_, :], o1p[m_][:], AF.Silu, bias=ob1t[:, m_:m_ + 1])
                hout = sb.tile([4, VOCAB], dt.float32, tag="hout", name="hout")
                for mo in range(7):
                    mw = min(128, VOCAB - mo * 128)
                    o2p = ps.tile([128, 4], dt.float32, tag="small", name="small")
                    for k in range(2):
                        mm(o2p[:mw, :], oW2t[:, k, mo * 128:mo * 128 + mw], o1[:, k, :],
                           k == 0, k == 1)
                    o2b = sb.tile([128, 4], dt.float32, tag="o2b", name="o2b")
                    act(o2b[:mw, :], o2p[:mw, :], AF.Identity, bias=ob2t[:mw, mo:mo + 1])
                    fp = ps.tile([4, 128], dt.float32, tag="small", name="small")
                    nc.tensor.transpose(out=fp[:, :mw], in_=o2b[:mw, :],
                                        identity=ident[:mw, :mw])
                    nc.vector.tensor_copy(hout[:, mo * 128:mo * 128 + mw], fp[:, :mw])
                nc.sync.dma_start(head_out[:], hout[:])
            else:
                zt = sb.tile([4, VOCAB], dt.float32, tag="zt", name="zt")
                nc.gpsimd.memset(zt[:], 0.0)
                nc.sync.dma_start(head_out[:], zt[:])

    nc.compile()
    return nc


def build_and_run(inputs, nl=N_LAYERS, with_head=True, dbg=(), trace=False):
    from concourse.bass_utils import run_bass_kernel_spmd
    meta, maps = _prep(inputs)
    key = (nl, with_head, tuple(dbg))
    if key not in _cache:
        _cache[key] = _build(meta, nl=nl, with_head=with_head, dbg=dbg)
    nc = _cache[key]
    res = run_bass_kernel_spmd(nc, maps, core_ids=list(range(NC)), trace=trace)
    return res


def decode_state(arr):
    """dbg [128, 2*NS + G*4] -> (h [4096, 256], x [4096, 3]) node-local order."""
    hT = arr[:, :2 * NS].reshape(128, 2, NS)
    h = np.concatenate([hT[:, 0, :].T, hT[:, 1, :].T], 1)  # [NS, 256] w/ node = col
    xn = arr[:, 2 * NS:].reshape(128, G, 4)
    x = xn.transpose(1, 0, 2).reshape(NS, 4)[:, :3]
    return h, x


def kernel(**inputs) -> np.ndarray:
    res = build_and_run(inputs)
    out = np.concatenate([res.results[c]["head_out"] for c in range(NC)], 0)
    return out.astype(np.float32)
